# revision 44
# baseline (speedup 1.0000x reference)
"""GCN (2x GCNConv + linear head) on 8 NeuronCores via Bass/Tile.

v2 strategy (graph/data parallel per sharding hint, PE-centric):
  - Nodes padded to 10240 = 80 blocks of 128; core c owns dst range
    [c*1280, (c+1)*1280).
  - S = D^-1/2 (A+I) D^-1/2 factored so the sparse matrix holds exact
    small-integer counts stored as dense 128-row fp8 blocks (exact 0/1/2),
    contracted on the TensorEngine; per-node rsqrt scalings applied on
    DVE/ACT on device.
  - Layer algebra reassociated: acc = (A+I)^T (Dinv x);  z = W^T acc;
    h = relu(Dinv z + b) — so the weight matmul has a single resident
    stationary and the aggregation contracts raw scaled features.
  - Layer 1 runs dst-chunk-outer (512/512/256): each chunk's AllGather
    fires as soon as that chunk is done, overlapping comms with the next
    chunk's matmuls. The gathered shards return via transpose-DMA into
    node-major blocks for layer 2, which consumes src blocks in
    chunk-arrival order.
  - Head (h2 @ Wh + bh) in f32; output shard [1280, 40]; host trims pads.
"""
import numpy as np
import ml_dtypes

import concourse.bass as bass
import concourse.mybir as mybir
import concourse.tile as tile
import concourse.bacc as bacc
from concourse.bass_utils import run_bass_kernel_spmd

FP8 = np.dtype(ml_dtypes.float8_e4m3)
BF16 = np.dtype(ml_dtypes.bfloat16)

N, E, D, C = 10000, 640000, 128, 40
NCORES = 8
NSB = 80                      # src blocks of 128
NPAD = NSB * 128              # 10240
DST = NPAD // NCORES          # 1280 dst nodes per core
CH = [(0, 512), (512, 512), (1024, 256)]
NHK = DST // 128              # head chunks of 128

_cache = {}


def _build():
    nc = bacc.Bacc("TRN2", target_bir_lowering=False, debug=False,
                   num_devices=NCORES)
    f32 = mybir.dt.float32
    bf16 = mybir.dt.bfloat16
    fp8 = mybir.dt.float8e4
    RELU = mybir.ActivationFunctionType.Relu

    x_nm = nc.dram_tensor("x_nm", [128, NPAD], bf16, kind="ExternalInput")
    W1b = nc.dram_tensor("W1b", [D, D], bf16, kind="ExternalInput")
    W2b = nc.dram_tensor("W2b", [D, D], bf16, kind="ExternalInput")
    Wh = nc.dram_tensor("Wh", [D, C], f32, kind="ExternalInput")
    b1 = nc.dram_tensor("b1", [D, 1], f32, kind="ExternalInput")
    b2 = nc.dram_tensor("b2", [D, 1], f32, kind="ExternalInput")
    bh = nc.dram_tensor("bh", [1, C], f32, kind="ExternalInput")
    eye = nc.dram_tensor("eye", [128, 128], bf16, kind="ExternalInput")
    degc = nc.dram_tensor("degc", [128, NSB], f32, kind="ExternalInput")
    degs = nc.dram_tensor("degs", [1, DST], f32, kind="ExternalInput")
    A_d = [nc.dram_tensor(f"A{ci}", [128, NSB * ln], fp8, kind="ExternalInput")
           for ci, (off, ln) in enumerate(CH)]
    out = nc.dram_tensor("out", [DST, C], f32, kind="ExternalOutput")

    with tile.TileContext(nc) as tc:
        with (
            tc.tile_pool(name="big", bufs=1) as big,
            tc.tile_pool(name="sb", bufs=1) as sb,
            tc.tile_pool(name="tmpp", bufs=3) as tmpp,
            tc.tile_pool(name="psagg", bufs=3, space="PSUM") as psagg,
            tc.tile_pool(name="psz", bufs=1, space="PSUM") as psz,
            tc.tile_pool(name="pstr", bufs=2, space="PSUM") as pstr,
            tc.tile_pool(name="psmisc", bufs=1, space="PSUM") as psmisc,
            tc.tile_pool(name="dram", bufs=1, space="DRAM") as dram,
        ):
            # ---- warm-up collective: absorbs the cross-core entry barrier
            # and comm bootstrap under layer-1 compute. The input is never
            # written (content irrelevant), so the trigger has no deps and
            # fires at the top of the program on every core ----
            warm_in = dram.tile([1, 128], bf16)
            warm_out = dram.tile([NCORES, 1, 128], bf16, addr_space="Shared")
            nc.gpsimd.collective_compute(
                "AllGather", mybir.AluOpType.bypass,
                replica_groups=[list(range(NCORES))],
                ins=[warm_in[:]], outs=[warm_out[:]])

            # ---- small inputs first ----
            degc_t = sb.tile([128, NSB], f32)
            nc.sync.dma_start(degc_t[:], degc[:, :])
            degs_t = sb.tile([1, DST], f32)
            nc.sync.dma_start(degs_t[:], degs[:, :])
            W1_t = sb.tile([D, D], bf16)
            nc.sync.dma_start(W1_t[:], W1b[:, :])
            W2_t = sb.tile([D, D], bf16)
            nc.sync.dma_start(W2_t[:], W2b[:, :])
            Wh_t = sb.tile([D, C], f32)
            nc.sync.dma_start(Wh_t[:], Wh[:, :])
            b1_t = sb.tile([D, 1], f32)
            nc.sync.dma_start(b1_t[:], b1[:, :])
            b2_t = sb.tile([D, 1], f32)
            nc.sync.dma_start(b2_t[:], b2[:, :])
            bh_t = sb.tile([1, C], f32)
            nc.sync.dma_start(bh_t[:], bh[:, :])
            eye_t = sb.tile([128, 128], bf16)
            nc.sync.dma_start(eye_t[:], eye[:, :])

            # ---- big streams ----
            A_t = []
            for ci, (off, ln) in enumerate(CH):
                a = big.tile([128, NSB * ln], fp8, name=f"At{ci}")
                A_t.append(a)
            # A streams on the Sync HWDGE queue, in consumption order;
            # pieced so the agg matmuls unblock progressively.
            for ci, (off, ln) in enumerate(CH):
                npc = 8 if ci == 0 else 4
                for q in range(npc):
                    s0 = q * (NSB // npc) * ln
                    s1 = (q + 1) * (NSB // npc) * ln
                    nc.sync.dma_start(A_t[ci][:, s0:s1], A_d[ci][:, s0:s1])
            NPC = 8

            # ---- per-node scalings ----
            dinvc = sb.tile([128, NSB], f32)
            nc.vector.reciprocal(dinvc[:], degc_t[:])
            nc.scalar.sqrt(dinvc[:], dinvc[:])

            ones = sb.tile([1, 128], f32)
            nc.any.memset(ones[:], 1.0)
            dinvb = sb.tile([128, DST], f32)
            for off, ln in CH:
                ob = psmisc.tile([128, 512], f32, tag="outer")
                nc.tensor.matmul(ob[:, :ln], lhsT=ones[:],
                                 rhs=degs_t[:, off:off + ln],
                                 start=True, stop=True)
                nc.vector.reciprocal(dinvb[:, off:off + ln], ob[:, :ln])
                nc.scalar.sqrt(dinvb[:, off:off + ln], dinvb[:, off:off + ln])
            bhb = sb.tile([128, C], f32)
            obh = psmisc.tile([128, 512], f32, tag="outer")
            nc.tensor.matmul(obh[:, :C], lhsT=ones[:], rhs=bh_t[:, :],
                             start=True, stop=True)
            nc.vector.tensor_copy(bhb[:], obh[:, :C])

            # ---- g0 = Dinv * x, node-major bf16 (x streamed in pieces) ----
            g0 = big.tile([128, NPAD], bf16)
            PSB = NSB // NPC
            for p in range(NPC):
                sb0 = p * PSB
                xp = tmpp.tile([128, PSB * 128], bf16, tag="xp", bufs=4)
                # scalar HWDGE queue: runs concurrently with the A stream
                nc.scalar.dma_start(xp[:], x_nm[:, sb0 * 128:(sb0 + PSB) * 128])
                nc.vector.tensor_mul(
                    g0[:, sb0 * 128:(sb0 + PSB) * 128].rearrange(
                        "p (t f) -> p t f", f=128),
                    xp[:].rearrange("p (t f) -> p t f", f=128),
                    dinvc[:, sb0:sb0 + PSB].unsqueeze(2).broadcast_to(
                        [128, PSB, 128]))

            # ---- layer 1: chunk-outer aggregation; two allgathers
            # (512 early for L2 start, 768 merged to cut per-op overhead) ----
            g1full = sb.tile([128, DST], bf16)
            for ci, (off, ln) in enumerate(CH):
                agg = psagg.tile([128, 512], f32, tag="agg", name=f"agg1_{ci}")
                for sbk in range(NSB):
                    nc.tensor.matmul(
                        agg[:, :ln], lhsT=g0[:, sbk * 128:(sbk + 1) * 128],
                        rhs=A_t[ci][:, sbk * ln:(sbk + 1) * ln],
                        start=(sbk == 0), stop=(sbk == NSB - 1))
                acc = tmpp.tile([128, 512], bf16, tag="acc")
                nc.vector.tensor_copy(acc[:, :ln], agg[:, :ln])
                zps = psz.tile([128, 512], f32, tag="z")
                nc.tensor.matmul(zps[:, :ln], lhsT=W1_t[:], rhs=acc[:, :ln],
                                 start=True, stop=True)
                t1 = tmpp.tile([128, 512], f32, tag="t1")
                nc.vector.tensor_mul(t1[:, :ln], zps[:, :ln],
                                     dinvb[:, off:off + ln])
                t2 = tmpp.tile([128, 512], f32, tag="t2")
                nc.scalar.activation(t2[:, :ln], t1[:, :ln], RELU,
                                     bias=b1_t[:, 0:1], scale=1.0)
                nc.vector.tensor_mul(g1full[:, off:off + ln], t2[:, :ln],
                                     dinvb[:, off:off + ln])

            # cc segments: (dst offset, length) — fired when their g1 ready
            CCSEG = [(0, 512), (512, 768)]
            g1nm = []
            for gi, (goff, gln) in enumerate(CCSEG):
                cc_in = dram.tile([128, gln], bf16, name=f"cc_in{gi}")
                cc_out = dram.tile([NCORES, 128, gln], bf16,
                                   addr_space="Shared", name=f"cc_out{gi}")
                nc.scalar.dma_start(cc_in[:], g1full[:, goff:goff + gln])
                nc.gpsimd.collective_compute(
                    "AllGather", mybir.AluOpType.bypass,
                    replica_groups=[list(range(NCORES))],
                    ins=[cc_in[:]], outs=[cc_out[:]])
                # gather the slabs and transpose to node-major via PE
                # (transpose-DMA would serialize against in-flight collectives)
                nt = gln // 128
                gnm = big.tile([128, NCORES * gln], bf16, name=f"g1nm{gi}")
                slab = tmpp.tile([128, NCORES * 768], bf16, tag="slab", bufs=1)
                nc.sync.dma_start(
                    slab[:, :NCORES * gln].rearrange("p (r d) -> p r d", d=gln),
                    cc_out[:, :, :].rearrange("r p d -> p r d"))
                for r in range(NCORES):
                    for t0 in range(0, nt, 4):
                        tb = min(4, nt - t0)
                        trp = pstr.tile([128, 4, 128], bf16, tag="tr")
                        for t in range(t0, t0 + tb):
                            nc.tensor.transpose(
                                trp[:, t - t0, :],
                                slab[:, r * gln + t * 128:r * gln + (t + 1) * 128],
                                eye_t[:])
                        nc.vector.tensor_copy(
                            gnm[:, (r * nt + t0) * 128:(r * nt + t0 + tb) * 128]
                            .rearrange("p (t f) -> p t f", f=128),
                            trp[:, :tb, :])
                g1nm.append((gnm, nt))

            # ---- layer 2: src blocks in cc-segment-arrival order ----
            agg2 = [psagg.tile([128, 512], f32, tag="agg", name=f"agg2_{c2}")
                    for c2 in range(len(CH))]
            k = 0
            for gi, (goff, gln) in enumerate(CCSEG):
                gnm, nt = g1nm[gi]
                for r in range(NCORES):
                    for t in range(nt):
                        lhsT = gnm[:, (r * nt + t) * 128:(r * nt + t + 1) * 128]
                        sb_g = r * (DST // 128) + goff // 128 + t
                        for c2, (off2, ln2) in enumerate(CH):
                            nc.tensor.matmul(
                                agg2[c2][:, :ln2], lhsT=lhsT,
                                rhs=A_t[c2][:, sb_g * ln2:(sb_g + 1) * ln2],
                                start=(k == 0), stop=(k == NSB - 1))
                        k += 1

            h2 = sb.tile([128, DST], f32)
            for c2, (off2, ln2) in enumerate(CH):
                acc = tmpp.tile([128, 512], bf16, tag="acc")
                nc.vector.tensor_copy(acc[:, :ln2], agg2[c2][:, :ln2])
                zps = psz.tile([128, 512], f32, tag="z")
                nc.tensor.matmul(zps[:, :ln2], lhsT=W2_t[:], rhs=acc[:, :ln2],
                                 start=True, stop=True)
                t1 = tmpp.tile([128, 512], f32, tag="t1")
                nc.vector.tensor_mul(t1[:, :ln2], zps[:, :ln2],
                                     dinvb[:, off2:off2 + ln2])
                nc.scalar.activation(h2[:, off2:off2 + ln2], t1[:, :ln2],
                                     RELU, bias=b2_t[:, 0:1], scale=1.0)

            # ---- head ----
            out_sb = sb.tile([128, NHK * C], f32)
            for hk in range(NHK):
                hd = psmisc.tile([128, C], f32, tag="hd")
                nc.tensor.matmul(hd[:], lhsT=h2[:, hk * 128:(hk + 1) * 128],
                                 rhs=Wh_t[:], start=True, stop=True)
                nc.vector.tensor_add(out_sb[:, hk * C:(hk + 1) * C], hd[:],
                                     bhb[:, :])
            nc.scalar.dma_start(
                out[:, :].rearrange("(t p) c -> p t c", p=128),
                out_sb[:].rearrange("p (t c) -> p t c", c=C))
    nc.compile()
    return nc


def _prep(x, edge_index, W1, b1, W2, b2, Wh, bh):
    x = np.asarray(x, np.float32)
    ei = np.asarray(edge_index, np.int64)
    src = np.concatenate([ei[0], np.arange(NPAD, dtype=np.int64)])
    dst = np.concatenate([ei[1], np.arange(NPAD, dtype=np.int64)])
    deg = np.bincount(dst, minlength=NPAD).astype(np.float32)
    degc = deg.reshape(NSB, 128).T.copy()

    xp = np.zeros((NPAD, D), np.float32)
    xp[:N] = x
    x_nm = xp.reshape(NSB, 128, D).transpose(1, 0, 2).reshape(128, NPAD)

    shared = {
        "x_nm": x_nm.astype(BF16),
        "W1b": np.asarray(W1, np.float32).astype(BF16),
        "W2b": np.asarray(W2, np.float32).astype(BF16),
        "Wh": np.asarray(Wh, np.float32),
        "b1": np.asarray(b1, np.float32).reshape(D, 1),
        "b2": np.asarray(b2, np.float32).reshape(D, 1),
        "bh": np.asarray(bh, np.float32).reshape(1, C),
        "degc": degc,
        "eye": np.eye(128, dtype=np.float32).astype(BF16),
    }
    core = dst // DST
    sl, sbk = src % 128, src // 128
    in_maps = []
    for c in range(NCORES):
        m = core == c
        dloc = dst[m] - c * DST
        im = dict(shared, degs=deg[c * DST:(c + 1) * DST].reshape(1, DST))
        for ci, (off, ln) in enumerate(CH):
            m2 = (dloc >= off) & (dloc < off + ln)
            Ac = np.zeros((128, NSB * ln), np.float32)
            np.add.at(Ac, (sl[m][m2], sbk[m][m2] * ln + dloc[m2] - off), 1.0)
            im[f"A{ci}"] = Ac.astype(FP8)
        in_maps.append(im)
    return in_maps


def _run(inputs, trace=False):
    if "nc" not in _cache:
        _cache["nc"] = _build()
    in_maps = _prep(**inputs)
    res = run_bass_kernel_spmd(_cache["nc"], in_maps,
                               core_ids=list(range(NCORES)), trace=trace)
    out = np.concatenate([res.results[c]["out"] for c in range(NCORES)],
                         axis=0)[:N]
    return np.ascontiguousarray(out, dtype=np.float32), res


def kernel(**inputs):
    out, _ = _run(inputs, trace=False)
    return out


# revision 45
# speedup vs baseline: 1.0419x; 1.0419x over previous
"""GCN (2x GCNConv + linear head) on 8 NeuronCores via Bass/Tile.

v2 strategy (graph/data parallel per sharding hint, PE-centric):
  - Nodes padded to 10240 = 80 blocks of 128; core c owns dst range
    [c*1280, (c+1)*1280).
  - S = D^-1/2 (A+I) D^-1/2 factored so the sparse matrix holds exact
    small-integer counts stored as dense 128-row fp8 blocks (exact 0/1/2),
    contracted on the TensorEngine; per-node rsqrt scalings applied on
    DVE/ACT on device.
  - Layer algebra reassociated: acc = (A+I)^T (Dinv x);  z = W^T acc;
    h = relu(Dinv z + b) — so the weight matmul has a single resident
    stationary and the aggregation contracts raw scaled features.
  - Layer 1 runs dst-chunk-outer (512/512/256): each chunk's AllGather
    fires as soon as that chunk is done, overlapping comms with the next
    chunk's matmuls. The gathered shards return via transpose-DMA into
    node-major blocks for layer 2, which consumes src blocks in
    chunk-arrival order.
  - Head (h2 @ Wh + bh) in f32; output shard [1280, 40]; host trims pads.
"""
import numpy as np
import ml_dtypes

import concourse.bass as bass
import concourse.mybir as mybir
import concourse.tile as tile
import concourse.bacc as bacc
from concourse.bass_utils import run_bass_kernel_spmd

FP8 = np.dtype(ml_dtypes.float8_e4m3)
BF16 = np.dtype(ml_dtypes.bfloat16)

N, E, D, C = 10000, 640000, 128, 40
NCORES = 8
NSB = 80                      # src blocks of 128
NPAD = NSB * 128              # 10240
DST = NPAD // NCORES          # 1280 dst nodes per core
CH = [(0, 512), (512, 512), (1024, 256)]
NHK = DST // 128              # head chunks of 128

_cache = {}


def _build():
    nc = bacc.Bacc("TRN2", target_bir_lowering=False, debug=False,
                   num_devices=NCORES)
    f32 = mybir.dt.float32
    bf16 = mybir.dt.bfloat16
    fp8 = mybir.dt.float8e4
    RELU = mybir.ActivationFunctionType.Relu

    x_nm = nc.dram_tensor("x_nm", [128, NPAD], bf16, kind="ExternalInput")
    W1b = nc.dram_tensor("W1b", [D, D], bf16, kind="ExternalInput")
    W2b = nc.dram_tensor("W2b", [D, D], bf16, kind="ExternalInput")
    Wh = nc.dram_tensor("Wh", [D, C], f32, kind="ExternalInput")
    b1 = nc.dram_tensor("b1", [D, 1], f32, kind="ExternalInput")
    b2 = nc.dram_tensor("b2", [D, 1], f32, kind="ExternalInput")
    bh = nc.dram_tensor("bh", [1, C], f32, kind="ExternalInput")
    eye = nc.dram_tensor("eye", [128, 128], bf16, kind="ExternalInput")
    degc = nc.dram_tensor("degc", [128, NSB], f32, kind="ExternalInput")
    degs = nc.dram_tensor("degs", [1, DST], f32, kind="ExternalInput")
    A_d = [nc.dram_tensor(f"A{ci}", [128, NSB * ln], fp8, kind="ExternalInput")
           for ci, (off, ln) in enumerate(CH)]
    out = nc.dram_tensor("out", [DST, C], f32, kind="ExternalOutput")

    with tile.TileContext(nc) as tc:
        with (
            tc.tile_pool(name="big", bufs=1) as big,
            tc.tile_pool(name="sb", bufs=1) as sb,
            tc.tile_pool(name="tmpp", bufs=3) as tmpp,
            tc.tile_pool(name="psagg", bufs=3, space="PSUM") as psagg,
            tc.tile_pool(name="psz", bufs=1, space="PSUM") as psz,
            tc.tile_pool(name="pstr", bufs=2, space="PSUM") as pstr,
            tc.tile_pool(name="psmisc", bufs=1, space="PSUM") as psmisc,
            tc.tile_pool(name="dram", bufs=1, space="DRAM") as dram,
        ):
            # ---- warm-up collective: absorbs the cross-core entry barrier
            # and comm bootstrap under layer-1 compute. The input is never
            # written (content irrelevant), so the trigger has no deps and
            # fires at the top of the program on every core ----
            warm_in = dram.tile([1, 128], bf16)
            warm_out = dram.tile([NCORES, 1, 128], bf16, addr_space="Shared")
            nc.gpsimd.collective_compute(
                "AllGather", mybir.AluOpType.bypass,
                replica_groups=[list(range(NCORES))],
                ins=[warm_in[:]], outs=[warm_out[:]])

            # ---- small inputs first ----
            degc_t = sb.tile([128, NSB], f32)
            nc.sync.dma_start(degc_t[:], degc[:, :])
            degs_t = sb.tile([1, DST], f32)
            nc.sync.dma_start(degs_t[:], degs[:, :])
            W1_t = sb.tile([D, D], bf16)
            nc.sync.dma_start(W1_t[:], W1b[:, :])
            W2_t = sb.tile([D, D], bf16)
            nc.sync.dma_start(W2_t[:], W2b[:, :])
            Wh_t = sb.tile([D, C], f32)
            nc.sync.dma_start(Wh_t[:], Wh[:, :])
            b1_t = sb.tile([D, 1], f32)
            nc.sync.dma_start(b1_t[:], b1[:, :])
            b2_t = sb.tile([D, 1], f32)
            nc.sync.dma_start(b2_t[:], b2[:, :])
            bh_t = sb.tile([1, C], f32)
            nc.sync.dma_start(bh_t[:], bh[:, :])
            eye_t = sb.tile([128, 128], bf16)
            nc.sync.dma_start(eye_t[:], eye[:, :])

            # ---- big streams ----
            A_t = []
            for ci, (off, ln) in enumerate(CH):
                a = big.tile([128, NSB * ln], fp8, name=f"At{ci}")
                A_t.append(a)
            # A streams on the Sync HWDGE queue, in consumption order;
            # pieced so the agg matmuls unblock progressively.
            for ci, (off, ln) in enumerate(CH):
                npc = 8 if ci == 0 else 4
                for q in range(npc):
                    s0 = q * (NSB // npc) * ln
                    s1 = (q + 1) * (NSB // npc) * ln
                    nc.sync.dma_start(A_t[ci][:, s0:s1], A_d[ci][:, s0:s1])
            NPC = 8

            # ---- per-node scalings ----
            dinvc = sb.tile([128, NSB], f32)
            nc.vector.reciprocal(dinvc[:], degc_t[:])
            nc.scalar.sqrt(dinvc[:], dinvc[:])

            ones = sb.tile([1, 128], f32)
            nc.any.memset(ones[:], 1.0)
            dinvb = sb.tile([128, DST], f32)
            for off, ln in CH:
                ob = psmisc.tile([128, 512], f32, tag="outer")
                nc.tensor.matmul(ob[:, :ln], lhsT=ones[:],
                                 rhs=degs_t[:, off:off + ln],
                                 start=True, stop=True)
                nc.vector.reciprocal(dinvb[:, off:off + ln], ob[:, :ln])
                nc.scalar.sqrt(dinvb[:, off:off + ln], dinvb[:, off:off + ln])
            bhb = sb.tile([128, C], f32)
            obh = psmisc.tile([128, 512], f32, tag="outer")
            nc.tensor.matmul(obh[:, :C], lhsT=ones[:], rhs=bh_t[:, :],
                             start=True, stop=True)
            nc.vector.tensor_copy(bhb[:], obh[:, :C])

            # ---- g0 = Dinv * x, node-major bf16 (x streamed in pieces) ----
            g0 = big.tile([128, NPAD], bf16)
            PSB = NSB // NPC
            for p in range(NPC):
                sb0 = p * PSB
                xp = tmpp.tile([128, PSB * 128], bf16, tag="xp", bufs=4)
                # scalar HWDGE queue: runs concurrently with the A stream
                nc.scalar.dma_start(xp[:], x_nm[:, sb0 * 128:(sb0 + PSB) * 128])
                nc.vector.tensor_mul(
                    g0[:, sb0 * 128:(sb0 + PSB) * 128].rearrange(
                        "p (t f) -> p t f", f=128),
                    xp[:].rearrange("p (t f) -> p t f", f=128),
                    dinvc[:, sb0:sb0 + PSB].unsqueeze(2).broadcast_to(
                        [128, PSB, 128]))

            # ---- layer 1: chunk-outer aggregation + pipelined allgather ----
            g1nm = []       # node-major gathered g1, one tensor per chunk
            cc_pairs = []
            for ci, (off, ln) in enumerate(CH):
                agg = psagg.tile([128, 512], f32, tag="agg", name=f"agg1_{ci}")
                for sbk in range(NSB):
                    nc.tensor.matmul(
                        agg[:, :ln], lhsT=g0[:, sbk * 128:(sbk + 1) * 128],
                        rhs=A_t[ci][:, sbk * ln:(sbk + 1) * ln],
                        start=(sbk == 0), stop=(sbk == NSB - 1))
                acc = tmpp.tile([128, 512], bf16, tag="acc")
                nc.vector.tensor_copy(acc[:, :ln], agg[:, :ln])
                zps = psz.tile([128, 512], f32, tag="z")
                nc.tensor.matmul(zps[:, :ln], lhsT=W1_t[:], rhs=acc[:, :ln],
                                 start=True, stop=True)
                t1 = tmpp.tile([128, 512], f32, tag="t1")
                nc.vector.tensor_mul(t1[:, :ln], zps[:, :ln],
                                     dinvb[:, off:off + ln])
                t2 = tmpp.tile([128, 512], f32, tag="t2")
                nc.scalar.activation(t2[:, :ln], t1[:, :ln], RELU,
                                     bias=b1_t[:, 0:1], scale=1.0)
                g1c = tmpp.tile([128, 512], bf16, tag="g1c")
                nc.vector.tensor_mul(g1c[:, :ln], t2[:, :ln],
                                     dinvb[:, off:off + ln])
                # fire this chunk's allgather
                cc_in = dram.tile([128, ln], bf16, name=f"cc_in{ci}")
                cc_out = dram.tile([NCORES, 128, ln], bf16,
                                   addr_space="Shared", name=f"cc_out{ci}")
                nc.scalar.dma_start(cc_in[:], g1c[:, :ln])
                nc.gpsimd.collective_compute(
                    "AllGather", mybir.AluOpType.bypass,
                    replica_groups=[list(range(NCORES))],
                    ins=[cc_in[:]], outs=[cc_out[:]])
                # gather the slabs and transpose to node-major via PE
                # (transpose-DMA would serialize against in-flight collectives)
                nt = ln // 128
                gnm = big.tile([128, NCORES * ln], bf16, name=f"g1nm{ci}")
                slab = tmpp.tile([128, NCORES * 512], bf16, tag="slab", bufs=1)
                nc.sync.dma_start(
                    slab[:, :NCORES * ln].rearrange("p (r d) -> p r d", d=ln),
                    cc_out[:, :, :].rearrange("r p d -> p r d"))
                for r in range(NCORES):
                    for t0 in range(0, nt, 4):
                        tb = min(4, nt - t0)
                        trp = pstr.tile([128, 4, 128], bf16, tag="tr")
                        for t in range(t0, t0 + tb):
                            nc.tensor.transpose(
                                trp[:, t - t0, :],
                                slab[:, r * ln + t * 128:r * ln + (t + 1) * 128],
                                eye_t[:])
                        nc.vector.tensor_copy(
                            gnm[:, (r * nt + t0) * 128:(r * nt + t0 + tb) * 128]
                            .rearrange("p (t f) -> p t f", f=128),
                            trp[:, :tb, :])
                g1nm.append((gnm, nt))
                cc_pairs.append((cc_in, cc_out))

            # ---- layer 2: src blocks in chunk-arrival order ----
            agg2 = [psagg.tile([128, 512], f32, tag="agg", name=f"agg2_{c2}")
                    for c2 in range(len(CH))]
            k = 0
            for ci, (off, ln) in enumerate(CH):
                gnm, nt = g1nm[ci]
                for r in range(NCORES):
                    for t in range(nt):
                        lhsT = gnm[:, (r * nt + t) * 128:(r * nt + t + 1) * 128]
                        sb_g = r * (DST // 128) + off // 128 + t
                        for c2, (off2, ln2) in enumerate(CH):
                            nc.tensor.matmul(
                                agg2[c2][:, :ln2], lhsT=lhsT,
                                rhs=A_t[c2][:, sb_g * ln2:(sb_g + 1) * ln2],
                                start=(k == 0), stop=(k == NSB - 1))
                        k += 1

            h2 = sb.tile([128, DST], f32)
            for c2, (off2, ln2) in enumerate(CH):
                acc = tmpp.tile([128, 512], bf16, tag="acc")
                nc.vector.tensor_copy(acc[:, :ln2], agg2[c2][:, :ln2])
                zps = psz.tile([128, 512], f32, tag="z")
                nc.tensor.matmul(zps[:, :ln2], lhsT=W2_t[:], rhs=acc[:, :ln2],
                                 start=True, stop=True)
                t1 = tmpp.tile([128, 512], f32, tag="t1")
                nc.vector.tensor_mul(t1[:, :ln2], zps[:, :ln2],
                                     dinvb[:, off2:off2 + ln2])
                nc.scalar.activation(h2[:, off2:off2 + ln2], t1[:, :ln2],
                                     RELU, bias=b2_t[:, 0:1], scale=1.0)

            # ---- head ----
            out_sb = sb.tile([128, NHK * C], f32)
            for hk in range(NHK):
                hd = psmisc.tile([128, C], f32, tag="hd")
                nc.tensor.matmul(hd[:], lhsT=h2[:, hk * 128:(hk + 1) * 128],
                                 rhs=Wh_t[:], start=True, stop=True)
                nc.vector.tensor_add(out_sb[:, hk * C:(hk + 1) * C], hd[:],
                                     bhb[:, :])
            nc.scalar.dma_start(
                out[:, :].rearrange("(t p) c -> p t c", p=128),
                out_sb[:].rearrange("p (t c) -> p t c", c=C))
    nc.compile()
    return nc


def _prep(x, edge_index, W1, b1, W2, b2, Wh, bh):
    x = np.asarray(x, np.float32)
    ei = np.asarray(edge_index, np.int64)
    src = np.concatenate([ei[0], np.arange(NPAD, dtype=np.int64)])
    dst = np.concatenate([ei[1], np.arange(NPAD, dtype=np.int64)])
    deg = np.bincount(dst, minlength=NPAD).astype(np.float32)
    degc = deg.reshape(NSB, 128).T.copy()

    xp = np.zeros((NPAD, D), np.float32)
    xp[:N] = x
    x_nm = xp.reshape(NSB, 128, D).transpose(1, 0, 2).reshape(128, NPAD)

    shared = {
        "x_nm": x_nm.astype(BF16),
        "W1b": np.asarray(W1, np.float32).astype(BF16),
        "W2b": np.asarray(W2, np.float32).astype(BF16),
        "Wh": np.asarray(Wh, np.float32),
        "b1": np.asarray(b1, np.float32).reshape(D, 1),
        "b2": np.asarray(b2, np.float32).reshape(D, 1),
        "bh": np.asarray(bh, np.float32).reshape(1, C),
        "degc": degc,
        "eye": np.eye(128, dtype=np.float32).astype(BF16),
    }
    core = dst // DST
    sl, sbk = src % 128, src // 128
    in_maps = []
    for c in range(NCORES):
        m = core == c
        dloc = dst[m] - c * DST
        im = dict(shared, degs=deg[c * DST:(c + 1) * DST].reshape(1, DST))
        for ci, (off, ln) in enumerate(CH):
            m2 = (dloc >= off) & (dloc < off + ln)
            Ac = np.zeros((128, NSB * ln), np.float32)
            np.add.at(Ac, (sl[m][m2], sbk[m][m2] * ln + dloc[m2] - off), 1.0)
            im[f"A{ci}"] = Ac.astype(FP8)
        in_maps.append(im)
    return in_maps


def _run(inputs, trace=False):
    if "nc" not in _cache:
        _cache["nc"] = _build()
    in_maps = _prep(**inputs)
    res = run_bass_kernel_spmd(_cache["nc"], in_maps,
                               core_ids=list(range(NCORES)), trace=trace)
    out = np.concatenate([res.results[c]["out"] for c in range(NCORES)],
                         axis=0)[:N]
    return np.ascontiguousarray(out, dtype=np.float32), res


def kernel(**inputs):
    out, _ = _run(inputs, trace=False)
    return out


# revision 47
# speedup vs baseline: 1.0475x; 1.0054x over previous
"""GCN (2x GCNConv + linear head) on 8 NeuronCores via Bass/Tile.

v2 strategy (graph/data parallel per sharding hint, PE-centric):
  - Nodes padded to 10240 = 80 blocks of 128; core c owns dst range
    [c*1280, (c+1)*1280).
  - S = D^-1/2 (A+I) D^-1/2 factored so the sparse matrix holds exact
    small-integer counts stored as dense 128-row fp8 blocks (exact 0/1/2),
    contracted on the TensorEngine; per-node rsqrt scalings applied on
    DVE/ACT on device.
  - Layer algebra reassociated: acc = (A+I)^T (Dinv x);  z = W^T acc;
    h = relu(Dinv z + b) — so the weight matmul has a single resident
    stationary and the aggregation contracts raw scaled features.
  - Layer 1 runs dst-chunk-outer (512/512/256): each chunk's AllGather
    fires as soon as that chunk is done, overlapping comms with the next
    chunk's matmuls. The gathered shards return via transpose-DMA into
    node-major blocks for layer 2, which consumes src blocks in
    chunk-arrival order.
  - Head (h2 @ Wh + bh) in f32; output shard [1280, 40]; host trims pads.
"""
import numpy as np
import ml_dtypes

import concourse.bass as bass
import concourse.mybir as mybir
import concourse.tile as tile
import concourse.bacc as bacc
from concourse.bass_utils import run_bass_kernel_spmd

FP8 = np.dtype(ml_dtypes.float8_e4m3)
BF16 = np.dtype(ml_dtypes.bfloat16)

N, E, D, C = 10000, 640000, 128, 40
NCORES = 8
NSB = 80                      # src blocks of 128
NPAD = NSB * 128              # 10240
DST = NPAD // NCORES          # 1280 dst nodes per core
CH = [(0, 512), (512, 512), (1024, 256)]
NHK = DST // 128              # head chunks of 128

_cache = {}


def _build():
    nc = bacc.Bacc("TRN2", target_bir_lowering=False, debug=False,
                   num_devices=NCORES)
    f32 = mybir.dt.float32
    bf16 = mybir.dt.bfloat16
    fp8 = mybir.dt.float8e4
    RELU = mybir.ActivationFunctionType.Relu

    x_nm = nc.dram_tensor("x_nm", [128, NPAD], bf16, kind="ExternalInput")
    W1b = nc.dram_tensor("W1b", [D, D], bf16, kind="ExternalInput")
    W2b = nc.dram_tensor("W2b", [D, D], bf16, kind="ExternalInput")
    Wh = nc.dram_tensor("Wh", [D, C], f32, kind="ExternalInput")
    b1 = nc.dram_tensor("b1", [D, 1], f32, kind="ExternalInput")
    b2 = nc.dram_tensor("b2", [D, 1], f32, kind="ExternalInput")
    bh = nc.dram_tensor("bh", [1, C], f32, kind="ExternalInput")
    eye = nc.dram_tensor("eye", [128, 128], bf16, kind="ExternalInput")
    degc = nc.dram_tensor("degc", [128, NSB], f32, kind="ExternalInput")
    degs = nc.dram_tensor("degs", [1, DST], f32, kind="ExternalInput")
    A_d = [nc.dram_tensor(f"A{ci}", [128, NSB * ln], fp8, kind="ExternalInput")
           for ci, (off, ln) in enumerate(CH)]
    out = nc.dram_tensor("out", [DST, C], f32, kind="ExternalOutput")

    with tile.TileContext(nc) as tc:
        with (
            tc.tile_pool(name="big", bufs=1) as big,
            tc.tile_pool(name="sb", bufs=1) as sb,
            tc.tile_pool(name="tmpp", bufs=3) as tmpp,
            tc.tile_pool(name="psagg", bufs=3, space="PSUM") as psagg,
            tc.tile_pool(name="psz", bufs=1, space="PSUM") as psz,
            tc.tile_pool(name="pstr", bufs=2, space="PSUM") as pstr,
            tc.tile_pool(name="psmisc", bufs=1, space="PSUM") as psmisc,
            tc.tile_pool(name="dram", bufs=1, space="DRAM") as dram,
        ):
            # ---- warm-up collective: absorbs the cross-core entry barrier
            # and comm bootstrap under layer-1 compute. The input is never
            # written (content irrelevant), so the trigger has no deps and
            # fires at the top of the program on every core ----
            warm_in = dram.tile([1, 128], bf16)
            warm_out = dram.tile([NCORES, 1, 128], bf16, addr_space="Shared")
            nc.gpsimd.collective_compute(
                "AllGather", mybir.AluOpType.bypass,
                replica_groups=[list(range(NCORES))],
                ins=[warm_in[:]], outs=[warm_out[:]])

            # ---- small inputs first ----
            degc_t = sb.tile([128, NSB], f32)
            nc.sync.dma_start(degc_t[:], degc[:, :])
            degs_t = sb.tile([1, DST], f32)
            nc.sync.dma_start(degs_t[:], degs[:, :])
            W1_t = sb.tile([D, D], bf16)
            nc.sync.dma_start(W1_t[:], W1b[:, :])
            W2_t = sb.tile([D, D], bf16)
            nc.sync.dma_start(W2_t[:], W2b[:, :])
            Wh_t = sb.tile([D, C], f32)
            nc.sync.dma_start(Wh_t[:], Wh[:, :])
            b1_t = sb.tile([D, 1], f32)
            nc.sync.dma_start(b1_t[:], b1[:, :])
            b2_t = sb.tile([D, 1], f32)
            nc.sync.dma_start(b2_t[:], b2[:, :])
            bh_t = sb.tile([1, C], f32)
            nc.sync.dma_start(bh_t[:], bh[:, :])
            eye_t = sb.tile([128, 128], bf16)
            nc.sync.dma_start(eye_t[:], eye[:, :])

            # ---- big streams ----
            A_t = []
            for ci, (off, ln) in enumerate(CH):
                a = big.tile([128, NSB * ln], fp8, name=f"At{ci}")
                A_t.append(a)
            # A streams on the Sync HWDGE queue, in consumption order;
            # pieced so the agg matmuls unblock progressively.
            for ci, (off, ln) in enumerate(CH):
                npc = 8 if ci == 0 else 4
                for q in range(npc):
                    s0 = q * (NSB // npc) * ln
                    s1 = (q + 1) * (NSB // npc) * ln
                    nc.sync.dma_start(A_t[ci][:, s0:s1], A_d[ci][:, s0:s1])
            NPC = 8

            # ---- per-node scalings ----
            dinvc = sb.tile([128, NSB], f32)
            nc.vector.reciprocal(dinvc[:], degc_t[:])
            nc.scalar.sqrt(dinvc[:], dinvc[:])

            ones = sb.tile([1, 128], f32)
            nc.any.memset(ones[:], 1.0)
            dinvb = sb.tile([128, DST], f32)
            for off, ln in CH:
                ob = psmisc.tile([128, 512], f32, tag="outer")
                nc.tensor.matmul(ob[:, :ln], lhsT=ones[:],
                                 rhs=degs_t[:, off:off + ln],
                                 start=True, stop=True)
                nc.vector.reciprocal(dinvb[:, off:off + ln], ob[:, :ln])
                nc.scalar.sqrt(dinvb[:, off:off + ln], dinvb[:, off:off + ln])
            bhb = sb.tile([128, C], f32)
            obh = psmisc.tile([128, 512], f32, tag="outer")
            nc.tensor.matmul(obh[:, :C], lhsT=ones[:], rhs=bh_t[:, :],
                             start=True, stop=True)
            nc.vector.tensor_copy(bhb[:], obh[:, :C])

            # ---- g0 = Dinv * x, node-major bf16 (x streamed in pieces) ----
            g0 = big.tile([128, NPAD], bf16)
            PSB = NSB // NPC
            for p in range(NPC):
                sb0 = p * PSB
                xp = tmpp.tile([128, PSB * 128], bf16, tag="xp", bufs=4)
                # scalar HWDGE queue: runs concurrently with the A stream
                nc.scalar.dma_start(xp[:], x_nm[:, sb0 * 128:(sb0 + PSB) * 128])
                nc.vector.tensor_mul(
                    g0[:, sb0 * 128:(sb0 + PSB) * 128].rearrange(
                        "p (t f) -> p t f", f=128),
                    xp[:].rearrange("p (t f) -> p t f", f=128),
                    dinvc[:, sb0:sb0 + PSB].unsqueeze(2).broadcast_to(
                        [128, PSB, 128]))

            # ---- layer 1: chunk-outer aggregation + pipelined allgather ----
            g1nm = []       # node-major gathered g1, one tensor per chunk
            cc_pairs = []
            for ci, (off, ln) in enumerate(CH):
                agg = psagg.tile([128, 512], f32, tag="agg", name=f"agg1_{ci}")
                for sbk in range(NSB):
                    nc.tensor.matmul(
                        agg[:, :ln], lhsT=g0[:, sbk * 128:(sbk + 1) * 128],
                        rhs=A_t[ci][:, sbk * ln:(sbk + 1) * ln],
                        start=(sbk == 0), stop=(sbk == NSB - 1))
                acc = tmpp.tile([128, 512], bf16, tag="acc")
                nc.vector.tensor_copy(acc[:, :ln], agg[:, :ln])
                zps = psz.tile([128, 512], f32, tag="z")
                nc.tensor.matmul(zps[:, :ln], lhsT=W1_t[:], rhs=acc[:, :ln],
                                 start=True, stop=True)
                t1 = tmpp.tile([128, 512], f32, tag="t1")
                nc.vector.tensor_mul(t1[:, :ln], zps[:, :ln],
                                     dinvb[:, off:off + ln])
                t2 = tmpp.tile([128, 512], f32, tag="t2")
                nc.scalar.activation(t2[:, :ln], t1[:, :ln], RELU,
                                     bias=b1_t[:, 0:1], scale=1.0)
                g1c = tmpp.tile([128, 512], bf16, tag="g1c")
                nc.vector.tensor_mul(g1c[:, :ln], t2[:, :ln],
                                     dinvb[:, off:off + ln])
                # fire this chunk's allgather
                cc_in = dram.tile([128, ln], bf16, name=f"cc_in{ci}")
                cc_out = dram.tile([NCORES, 128, ln], bf16,
                                   addr_space="Shared", name=f"cc_out{ci}")
                nc.scalar.dma_start(cc_in[:], g1c[:, :ln])
                nc.gpsimd.collective_compute(
                    "AllGather", mybir.AluOpType.bypass,
                    replica_groups=[list(range(NCORES))],
                    ins=[cc_in[:]], outs=[cc_out[:]])
                # gather the slabs and transpose to node-major via PE
                # (transpose-DMA would serialize against in-flight collectives)
                nt = ln // 128
                gnm = big.tile([128, NCORES * ln], bf16, name=f"g1nm{ci}")
                slab = tmpp.tile([128, NCORES * 512], bf16, tag="slab", bufs=1)
                H = NCORES // 2
                for hh in range(2):
                    nc.sync.dma_start(
                        slab[:, hh * H * ln:(hh + 1) * H * ln].rearrange(
                            "p (r d) -> p r d", d=ln),
                        cc_out[hh * H:(hh + 1) * H, :, :].rearrange(
                            "r p d -> p r d"))
                for r in range(NCORES):
                    for t0 in range(0, nt, 4):
                        tb = min(4, nt - t0)
                        trp = pstr.tile([128, 4, 128], bf16, tag="tr")
                        for t in range(t0, t0 + tb):
                            nc.tensor.transpose(
                                trp[:, t - t0, :],
                                slab[:, r * ln + t * 128:r * ln + (t + 1) * 128],
                                eye_t[:])
                        nc.vector.tensor_copy(
                            gnm[:, (r * nt + t0) * 128:(r * nt + t0 + tb) * 128]
                            .rearrange("p (t f) -> p t f", f=128),
                            trp[:, :tb, :])
                g1nm.append((gnm, nt))
                cc_pairs.append((cc_in, cc_out))

            # ---- layer 2: src blocks in chunk-arrival order ----
            agg2 = [psagg.tile([128, 512], f32, tag="agg", name=f"agg2_{c2}")
                    for c2 in range(len(CH))]
            k = 0
            for ci, (off, ln) in enumerate(CH):
                gnm, nt = g1nm[ci]
                for r in range(NCORES):
                    for t in range(nt):
                        lhsT = gnm[:, (r * nt + t) * 128:(r * nt + t + 1) * 128]
                        sb_g = r * (DST // 128) + off // 128 + t
                        for c2, (off2, ln2) in enumerate(CH):
                            nc.tensor.matmul(
                                agg2[c2][:, :ln2], lhsT=lhsT,
                                rhs=A_t[c2][:, sb_g * ln2:(sb_g + 1) * ln2],
                                start=(k == 0), stop=(k == NSB - 1))
                        k += 1

            h2 = sb.tile([128, DST], f32)
            for c2, (off2, ln2) in enumerate(CH):
                acc = tmpp.tile([128, 512], bf16, tag="acc")
                nc.vector.tensor_copy(acc[:, :ln2], agg2[c2][:, :ln2])
                zps = psz.tile([128, 512], f32, tag="z")
                nc.tensor.matmul(zps[:, :ln2], lhsT=W2_t[:], rhs=acc[:, :ln2],
                                 start=True, stop=True)
                t1 = tmpp.tile([128, 512], f32, tag="t1")
                nc.vector.tensor_mul(t1[:, :ln2], zps[:, :ln2],
                                     dinvb[:, off2:off2 + ln2])
                nc.scalar.activation(h2[:, off2:off2 + ln2], t1[:, :ln2],
                                     RELU, bias=b2_t[:, 0:1], scale=1.0)

            # ---- head ----
            out_sb = sb.tile([128, NHK * C], f32)
            for hk in range(NHK):
                hd = psmisc.tile([128, C], f32, tag="hd")
                nc.tensor.matmul(hd[:], lhsT=h2[:, hk * 128:(hk + 1) * 128],
                                 rhs=Wh_t[:], start=True, stop=True)
                nc.vector.tensor_add(out_sb[:, hk * C:(hk + 1) * C], hd[:],
                                     bhb[:, :])
            HT = NHK // 2
            for hh in range(2):
                nc.scalar.dma_start(
                    out[hh * HT * 128:(hh + 1) * HT * 128, :].rearrange(
                        "(t p) c -> p t c", p=128),
                    out_sb[:, hh * HT * C:(hh + 1) * HT * C].rearrange(
                        "p (t c) -> p t c", c=C))
    nc.compile()
    return nc


def _prep(x, edge_index, W1, b1, W2, b2, Wh, bh):
    x = np.asarray(x, np.float32)
    ei = np.asarray(edge_index, np.int64)
    src = np.concatenate([ei[0], np.arange(NPAD, dtype=np.int64)])
    dst = np.concatenate([ei[1], np.arange(NPAD, dtype=np.int64)])
    deg = np.bincount(dst, minlength=NPAD).astype(np.float32)
    degc = deg.reshape(NSB, 128).T.copy()

    xp = np.zeros((NPAD, D), np.float32)
    xp[:N] = x
    x_nm = xp.reshape(NSB, 128, D).transpose(1, 0, 2).reshape(128, NPAD)

    shared = {
        "x_nm": x_nm.astype(BF16),
        "W1b": np.asarray(W1, np.float32).astype(BF16),
        "W2b": np.asarray(W2, np.float32).astype(BF16),
        "Wh": np.asarray(Wh, np.float32),
        "b1": np.asarray(b1, np.float32).reshape(D, 1),
        "b2": np.asarray(b2, np.float32).reshape(D, 1),
        "bh": np.asarray(bh, np.float32).reshape(1, C),
        "degc": degc,
        "eye": np.eye(128, dtype=np.float32).astype(BF16),
    }
    core = dst // DST
    sl, sbk = src % 128, src // 128
    in_maps = []
    for c in range(NCORES):
        m = core == c
        dloc = dst[m] - c * DST
        im = dict(shared, degs=deg[c * DST:(c + 1) * DST].reshape(1, DST))
        for ci, (off, ln) in enumerate(CH):
            m2 = (dloc >= off) & (dloc < off + ln)
            Ac = np.zeros((128, NSB * ln), np.float32)
            np.add.at(Ac, (sl[m][m2], sbk[m][m2] * ln + dloc[m2] - off), 1.0)
            im[f"A{ci}"] = Ac.astype(FP8)
        in_maps.append(im)
    return in_maps


def _run(inputs, trace=False):
    if "nc" not in _cache:
        _cache["nc"] = _build()
    in_maps = _prep(**inputs)
    res = run_bass_kernel_spmd(_cache["nc"], in_maps,
                               core_ids=list(range(NCORES)), trace=trace)
    out = np.concatenate([res.results[c]["out"] for c in range(NCORES)],
                         axis=0)[:N]
    return np.ascontiguousarray(out, dtype=np.float32), res


def kernel(**inputs):
    out, _ = _run(inputs, trace=False)
    return out


# revision 48
# speedup vs baseline: 1.0541x; 1.0064x over previous
"""GCN (2x GCNConv + linear head) on 8 NeuronCores via Bass/Tile.

v2 strategy (graph/data parallel per sharding hint, PE-centric):
  - Nodes padded to 10240 = 80 blocks of 128; core c owns dst range
    [c*1280, (c+1)*1280).
  - S = D^-1/2 (A+I) D^-1/2 factored so the sparse matrix holds exact
    small-integer counts stored as dense 128-row fp8 blocks (exact 0/1/2),
    contracted on the TensorEngine; per-node rsqrt scalings applied on
    DVE/ACT on device.
  - Layer algebra reassociated: acc = (A+I)^T (Dinv x);  z = W^T acc;
    h = relu(Dinv z + b) — so the weight matmul has a single resident
    stationary and the aggregation contracts raw scaled features.
  - Layer 1 runs dst-chunk-outer (512/512/256): each chunk's AllGather
    fires as soon as that chunk is done, overlapping comms with the next
    chunk's matmuls. The gathered shards return via transpose-DMA into
    node-major blocks for layer 2, which consumes src blocks in
    chunk-arrival order.
  - Head (h2 @ Wh + bh) in f32; output shard [1280, 40]; host trims pads.
"""
import numpy as np
import ml_dtypes

import concourse.bass as bass
import concourse.mybir as mybir
import concourse.tile as tile
import concourse.bacc as bacc
from concourse.bass_utils import run_bass_kernel_spmd

FP8 = np.dtype(ml_dtypes.float8_e4m3)
BF16 = np.dtype(ml_dtypes.bfloat16)

N, E, D, C = 10000, 640000, 128, 40
NCORES = 8
NSB = 80                      # src blocks of 128
NPAD = NSB * 128              # 10240
DST = NPAD // NCORES          # 1280 dst nodes per core
CH = [(0, 512), (512, 512), (1024, 256)]
NHK = DST // 128              # head chunks of 128

_cache = {}


def _build():
    nc = bacc.Bacc("TRN2", target_bir_lowering=False, debug=False,
                   num_devices=NCORES)
    f32 = mybir.dt.float32
    bf16 = mybir.dt.bfloat16
    fp8 = mybir.dt.float8e4
    RELU = mybir.ActivationFunctionType.Relu

    x_nm = nc.dram_tensor("x_nm", [128, NPAD], bf16, kind="ExternalInput")
    W1b = nc.dram_tensor("W1b", [D, D], bf16, kind="ExternalInput")
    W2b = nc.dram_tensor("W2b", [D, D], bf16, kind="ExternalInput")
    Wh = nc.dram_tensor("Wh", [D, C], f32, kind="ExternalInput")
    b1 = nc.dram_tensor("b1", [D, 1], f32, kind="ExternalInput")
    b2 = nc.dram_tensor("b2", [D, 1], f32, kind="ExternalInput")
    bh = nc.dram_tensor("bh", [1, C], f32, kind="ExternalInput")
    eye = nc.dram_tensor("eye", [128, 128], bf16, kind="ExternalInput")
    degc = nc.dram_tensor("degc", [128, NSB], f32, kind="ExternalInput")
    degs = nc.dram_tensor("degs", [1, DST], f32, kind="ExternalInput")
    A_d = [nc.dram_tensor(f"A{ci}", [128, NSB * ln], fp8, kind="ExternalInput")
           for ci, (off, ln) in enumerate(CH)]
    out = nc.dram_tensor("out", [DST, C], f32, kind="ExternalOutput")

    with tile.TileContext(nc) as tc:
        with (
            tc.tile_pool(name="big", bufs=1) as big,
            tc.tile_pool(name="sb", bufs=1) as sb,
            tc.tile_pool(name="tmpp", bufs=3) as tmpp,
            tc.tile_pool(name="psagg", bufs=3, space="PSUM") as psagg,
            tc.tile_pool(name="psz", bufs=1, space="PSUM") as psz,
            tc.tile_pool(name="pstr", bufs=2, space="PSUM") as pstr,
            tc.tile_pool(name="psmisc", bufs=1, space="PSUM") as psmisc,
            tc.tile_pool(name="dram", bufs=1, space="DRAM") as dram,
        ):
            # ---- warm-up collective: absorbs the cross-core entry barrier
            # and comm bootstrap under layer-1 compute. The input is never
            # written (content irrelevant), so the trigger has no deps and
            # fires at the top of the program on every core ----
            warm_in = dram.tile([1, 128], bf16)
            warm_out = dram.tile([NCORES, 1, 128], bf16, addr_space="Shared")
            nc.gpsimd.collective_compute(
                "AllGather", mybir.AluOpType.bypass,
                replica_groups=[list(range(NCORES))],
                ins=[warm_in[:]], outs=[warm_out[:]])

            # ---- small inputs first ----
            degc_t = sb.tile([128, NSB], f32)
            nc.sync.dma_start(degc_t[:], degc[:, :])
            degs_t = sb.tile([1, DST], f32)
            nc.sync.dma_start(degs_t[:], degs[:, :])
            W1_t = sb.tile([D, D], bf16)
            nc.sync.dma_start(W1_t[:], W1b[:, :])
            W2_t = sb.tile([D, D], bf16)
            nc.sync.dma_start(W2_t[:], W2b[:, :])
            Wh_t = sb.tile([D, C], f32)
            nc.sync.dma_start(Wh_t[:], Wh[:, :])
            b1_t = sb.tile([D, 1], f32)
            nc.sync.dma_start(b1_t[:], b1[:, :])
            b2_t = sb.tile([D, 1], f32)
            nc.sync.dma_start(b2_t[:], b2[:, :])
            bh_t = sb.tile([1, C], f32)
            nc.sync.dma_start(bh_t[:], bh[:, :])
            eye_t = sb.tile([128, 128], bf16)
            nc.sync.dma_start(eye_t[:], eye[:, :])

            # ---- big streams ----
            A_t = []
            for ci, (off, ln) in enumerate(CH):
                a = big.tile([128, NSB * ln], fp8, name=f"At{ci}")
                A_t.append(a)
            # A streams on the Sync HWDGE queue, in consumption order;
            # pieced so the agg matmuls unblock progressively.
            for ci, (off, ln) in enumerate(CH):
                npc = 8 if ci == 0 else 4
                for q in range(npc):
                    s0 = q * (NSB // npc) * ln
                    s1 = (q + 1) * (NSB // npc) * ln
                    nc.sync.dma_start(A_t[ci][:, s0:s1], A_d[ci][:, s0:s1])
            NPC = 8

            # ---- per-node scalings ----
            dinvc = sb.tile([128, NSB], f32)
            nc.vector.reciprocal(dinvc[:], degc_t[:])
            nc.scalar.sqrt(dinvc[:], dinvc[:])

            ones = sb.tile([1, 128], f32)
            nc.any.memset(ones[:], 1.0)
            dinvb = sb.tile([128, DST], f32)
            for off, ln in CH:
                ob = psmisc.tile([128, 512], f32, tag="outer")
                nc.tensor.matmul(ob[:, :ln], lhsT=ones[:],
                                 rhs=degs_t[:, off:off + ln],
                                 start=True, stop=True)
                nc.vector.reciprocal(dinvb[:, off:off + ln], ob[:, :ln])
                nc.scalar.sqrt(dinvb[:, off:off + ln], dinvb[:, off:off + ln])
            bhb = sb.tile([128, C], f32)
            obh = psmisc.tile([128, 512], f32, tag="outer")
            nc.tensor.matmul(obh[:, :C], lhsT=ones[:], rhs=bh_t[:, :],
                             start=True, stop=True)
            nc.vector.tensor_copy(bhb[:], obh[:, :C])

            # ---- g0 = Dinv * x, node-major bf16 (x streamed in pieces) ----
            g0 = big.tile([128, NPAD], bf16)
            PSB = NSB // NPC
            for p in range(NPC):
                sb0 = p * PSB
                xp = tmpp.tile([128, PSB * 128], bf16, tag="xp", bufs=4)
                # scalar HWDGE queue: runs concurrently with the A stream
                nc.scalar.dma_start(xp[:], x_nm[:, sb0 * 128:(sb0 + PSB) * 128])
                nc.vector.tensor_mul(
                    g0[:, sb0 * 128:(sb0 + PSB) * 128].rearrange(
                        "p (t f) -> p t f", f=128),
                    xp[:].rearrange("p (t f) -> p t f", f=128),
                    dinvc[:, sb0:sb0 + PSB].unsqueeze(2).broadcast_to(
                        [128, PSB, 128]))

            # ---- layer 1: chunk-outer aggregation + pipelined allgather ----
            g1nm = []       # node-major gathered g1, one tensor per chunk
            cc_pairs = []
            for ci, (off, ln) in enumerate(CH):
                agg = psagg.tile([128, 512], f32, tag="agg", name=f"agg1_{ci}")
                for sbk in range(NSB):
                    nc.tensor.matmul(
                        agg[:, :ln], lhsT=g0[:, sbk * 128:(sbk + 1) * 128],
                        rhs=A_t[ci][:, sbk * ln:(sbk + 1) * ln],
                        start=(sbk == 0), stop=(sbk == NSB - 1))
                acc = tmpp.tile([128, 512], bf16, tag="acc")
                nc.vector.tensor_copy(acc[:, :ln], agg[:, :ln])
                zps = psz.tile([128, 512], f32, tag="z")
                nc.tensor.matmul(zps[:, :ln], lhsT=W1_t[:], rhs=acc[:, :ln],
                                 start=True, stop=True)
                t1 = tmpp.tile([128, 512], f32, tag="t1")
                nc.vector.tensor_mul(t1[:, :ln], zps[:, :ln],
                                     dinvb[:, off:off + ln])
                t2 = tmpp.tile([128, 512], f32, tag="t2")
                nc.scalar.activation(t2[:, :ln], t1[:, :ln], RELU,
                                     bias=b1_t[:, 0:1], scale=1.0)
                g1c = tmpp.tile([128, 512], bf16, tag="g1c")
                nc.vector.tensor_mul(g1c[:, :ln], t2[:, :ln],
                                     dinvb[:, off:off + ln])
                # fire this chunk's allgather
                cc_in = dram.tile([128, ln], bf16, name=f"cc_in{ci}")
                cc_out = dram.tile([NCORES, 128, ln], bf16,
                                   addr_space="Shared", name=f"cc_out{ci}")
                nc.scalar.dma_start(cc_in[:], g1c[:, :ln])
                nc.gpsimd.collective_compute(
                    "AllGather", mybir.AluOpType.bypass,
                    replica_groups=[list(range(NCORES))],
                    ins=[cc_in[:]], outs=[cc_out[:]])
                # gather the slabs and transpose to node-major via PE
                # (transpose-DMA would serialize against in-flight collectives)
                nt = ln // 128
                gnm = big.tile([128, NCORES * ln], bf16, name=f"g1nm{ci}")
                slab = tmpp.tile([128, NCORES * 512], bf16, tag="slab", bufs=1)
                H = NCORES // 2
                for hh in range(2):
                    nc.sync.dma_start(
                        slab[:, hh * H * ln:(hh + 1) * H * ln].rearrange(
                            "p (r d) -> p r d", d=ln),
                        cc_out[hh * H:(hh + 1) * H, :, :].rearrange(
                            "r p d -> p r d"))
                for r in range(NCORES):
                    for t0 in range(0, nt, 4):
                        tb = min(4, nt - t0)
                        trp = pstr.tile([128, 4, 128], bf16, tag="tr")
                        for t in range(t0, t0 + tb):
                            nc.tensor.transpose(
                                trp[:, t - t0, :],
                                slab[:, r * ln + t * 128:r * ln + (t + 1) * 128],
                                eye_t[:])
                        dst_ap = gnm[:, (r * nt + t0) * 128:
                                     (r * nt + t0 + tb) * 128].rearrange(
                                         "p (t f) -> p t f", f=128)
                        # alternate DVE/ACT so the copies pipeline 2-wide
                        if r % 2 == 0:
                            nc.vector.tensor_copy(dst_ap, trp[:, :tb, :])
                        else:
                            nc.scalar.copy(dst_ap, trp[:, :tb, :])
                g1nm.append((gnm, nt))
                cc_pairs.append((cc_in, cc_out))

            # ---- layer 2: src blocks in chunk-arrival order ----
            agg2 = [psagg.tile([128, 512], f32, tag="agg", name=f"agg2_{c2}")
                    for c2 in range(len(CH))]
            k = 0
            for ci, (off, ln) in enumerate(CH):
                gnm, nt = g1nm[ci]
                for r in range(NCORES):
                    for t in range(nt):
                        lhsT = gnm[:, (r * nt + t) * 128:(r * nt + t + 1) * 128]
                        sb_g = r * (DST // 128) + off // 128 + t
                        for c2, (off2, ln2) in enumerate(CH):
                            nc.tensor.matmul(
                                agg2[c2][:, :ln2], lhsT=lhsT,
                                rhs=A_t[c2][:, sb_g * ln2:(sb_g + 1) * ln2],
                                start=(k == 0), stop=(k == NSB - 1))
                        k += 1

            h2 = sb.tile([128, DST], f32)
            for c2, (off2, ln2) in enumerate(CH):
                acc = tmpp.tile([128, 512], bf16, tag="acc")
                nc.vector.tensor_copy(acc[:, :ln2], agg2[c2][:, :ln2])
                zps = psz.tile([128, 512], f32, tag="z")
                nc.tensor.matmul(zps[:, :ln2], lhsT=W2_t[:], rhs=acc[:, :ln2],
                                 start=True, stop=True)
                t1 = tmpp.tile([128, 512], f32, tag="t1")
                nc.vector.tensor_mul(t1[:, :ln2], zps[:, :ln2],
                                     dinvb[:, off2:off2 + ln2])
                nc.scalar.activation(h2[:, off2:off2 + ln2], t1[:, :ln2],
                                     RELU, bias=b2_t[:, 0:1], scale=1.0)

            # ---- head ----
            out_sb = sb.tile([128, NHK * C], f32)
            for hk in range(NHK):
                hd = psmisc.tile([128, C], f32, tag="hd")
                nc.tensor.matmul(hd[:], lhsT=h2[:, hk * 128:(hk + 1) * 128],
                                 rhs=Wh_t[:], start=True, stop=True)
                nc.vector.tensor_add(out_sb[:, hk * C:(hk + 1) * C], hd[:],
                                     bhb[:, :])
            HT = NHK // 2
            for hh in range(2):
                nc.scalar.dma_start(
                    out[hh * HT * 128:(hh + 1) * HT * 128, :].rearrange(
                        "(t p) c -> p t c", p=128),
                    out_sb[:, hh * HT * C:(hh + 1) * HT * C].rearrange(
                        "p (t c) -> p t c", c=C))
    nc.compile()
    return nc


def _prep(x, edge_index, W1, b1, W2, b2, Wh, bh):
    x = np.asarray(x, np.float32)
    ei = np.asarray(edge_index, np.int64)
    src = np.concatenate([ei[0], np.arange(NPAD, dtype=np.int64)])
    dst = np.concatenate([ei[1], np.arange(NPAD, dtype=np.int64)])
    deg = np.bincount(dst, minlength=NPAD).astype(np.float32)
    degc = deg.reshape(NSB, 128).T.copy()

    xp = np.zeros((NPAD, D), np.float32)
    xp[:N] = x
    x_nm = xp.reshape(NSB, 128, D).transpose(1, 0, 2).reshape(128, NPAD)

    shared = {
        "x_nm": x_nm.astype(BF16),
        "W1b": np.asarray(W1, np.float32).astype(BF16),
        "W2b": np.asarray(W2, np.float32).astype(BF16),
        "Wh": np.asarray(Wh, np.float32),
        "b1": np.asarray(b1, np.float32).reshape(D, 1),
        "b2": np.asarray(b2, np.float32).reshape(D, 1),
        "bh": np.asarray(bh, np.float32).reshape(1, C),
        "degc": degc,
        "eye": np.eye(128, dtype=np.float32).astype(BF16),
    }
    core = dst // DST
    sl, sbk = src % 128, src // 128
    in_maps = []
    for c in range(NCORES):
        m = core == c
        dloc = dst[m] - c * DST
        im = dict(shared, degs=deg[c * DST:(c + 1) * DST].reshape(1, DST))
        for ci, (off, ln) in enumerate(CH):
            m2 = (dloc >= off) & (dloc < off + ln)
            Ac = np.zeros((128, NSB * ln), np.float32)
            np.add.at(Ac, (sl[m][m2], sbk[m][m2] * ln + dloc[m2] - off), 1.0)
            im[f"A{ci}"] = Ac.astype(FP8)
        in_maps.append(im)
    return in_maps


def _run(inputs, trace=False):
    if "nc" not in _cache:
        _cache["nc"] = _build()
    in_maps = _prep(**inputs)
    res = run_bass_kernel_spmd(_cache["nc"], in_maps,
                               core_ids=list(range(NCORES)), trace=trace)
    out = np.concatenate([res.results[c]["out"] for c in range(NCORES)],
                         axis=0)[:N]
    return np.ascontiguousarray(out, dtype=np.float32), res


def kernel(**inputs):
    out, _ = _run(inputs, trace=False)
    return out
